# revision 10
# baseline (speedup 1.0000x reference)
"""GCL encoder (2-layer GCN propagation) on 8 TRN2 NeuronCores.

Strategy: shard DESTINATION nodes across the 8 cores (12500 rows each);
replicate the [100000, 128] embedding table in every core's HBM. Each core
processes the ~400k edges whose destination row it owns:

  per layer:
    - dma_gather source rows x[col] from the (replicated) table, in 4
      column-blocks of 25000 rows so indices fit int16
    - build a one-hot scatter matrix per 128-edge tile on DVE:
      onehot[p, j] = (iota[j] == local_row[p]) * val[p]
    - TensorEngine matmul-accumulates onehot.T @ msgs into a per-window
      (128 dest rows) PSUM tile -> exact segment_sum
    - flush windows to an HBM shard buffer
  between layers: one AllGather assembles the full [100000, 128] layer-1
  output in every core's HBM (6.4 MB per rank).

Output: core c returns rows [c*12500, (c+1)*12500) of
(ego + e1 + e2) / 3; the host concatenates and splits into (sym, herb).
"""
import sys
import numpy as np

if '/opt/trn_rl_repo' not in sys.path:
    sys.path.insert(0, '/opt/trn_rl_repo')

# problem constants (hardcoded per spec)
NUM_SYM = 50000
NUM_HERB = 50000
N = NUM_SYM + NUM_HERB        # 100000 nodes
D = 128
N_CORES = 8
NSH = N // N_CORES            # 12500 dest rows per core
WIN = 128                     # dest-window rows (one PSUM tile)
NW = (NSH + WIN - 1) // WIN   # 98 windows (last has 84 rows)
NB = 4                        # column blocks (int16 gather index limit)
BLK = 25000
GRP = 4                       # windows per gather group
NG = (NW + GRP - 1) // GRP    # 25 groups
N_LAYERS = 2


def _set_config(n, nb, grp):
    """Test hook: rescale the geometry (keeps N_CORES=8, D=128)."""
    global N, NSH, NW, NB, BLK, NG
    N = n
    NSH = N // N_CORES
    NW = (NSH + WIN - 1) // WIN
    NB = nb
    BLK = N // NB
    NG = (NW + GRP - 1) // GRP if grp is None else (NW + grp - 1) // grp
    globals()['GRP'] = grp or GRP


def _preprocess(rows, cols, vals):
    """Bucket edges by (core, window, colblock); build per-core flat meta
    arrays (gather idx / local row / val) with shared-capacity padding so a
    single SPMD program fits all cores."""
    core = rows // NSH
    lrow = rows - core * NSH
    w = lrow // WIN
    rl = (lrow - w * WIN).astype(np.float32)
    b = cols // BLK
    colloc = (cols - b * BLK).astype(np.int16)

    key = ((core * NW + w) * NB + b).astype(np.int64)
    cnt = np.bincount(key, minlength=N_CORES * NW * NB).reshape(N_CORES, NW, NB)
    cap = cnt.max(axis=0)
    cap = ((cap + 127) // 128) * 128
    cap = np.maximum(cap, 128)            # [NW, NB]

    order = np.argsort(key, kind='stable')
    colloc_s = colloc[order]
    rl_s = rl[order]
    val_s = vals[order]
    # bucket start offsets in sorted arrays, per (core, w, b)
    starts = np.zeros(N_CORES * NW * NB + 1, np.int64)
    np.cumsum(cnt.ravel(), out=starts[1:])

    # processing order: for g, for b, for w in group — per-call layouts
    calls = []          # (g, b, L, [(w, ntiles)...])
    for g in range(NG):
        ws = list(range(g * GRP, min((g + 1) * GRP, NW)))
        for b in range(NB):
            L = int(sum(cap[w, b] for w in ws))
            calls.append((g, b, L, [(w, int(cap[w, b]) // 128) for w in ws]))

    idx_parts = [[] for _ in range(N_CORES)]
    rl_parts = [[] for _ in range(N_CORES)]
    val_parts = [[] for _ in range(N_CORES)]
    for c in range(N_CORES):
        for (g, b, L, wts) in calls:
            seg_i = np.zeros(L, np.int16)
            seg_r = np.zeros(L, np.float32)
            seg_v = np.zeros(L, np.float32)
            off = 0
            for (w, nt) in wts:
                k = (c * NW + w) * NB + b
                s, e = starts[k], starts[k + 1]
                n = e - s
                seg_i[off:off + n] = colloc_s[s:e]
                seg_r[off:off + n] = rl_s[s:e]
                seg_v[off:off + n] = val_s[s:e]
                off += nt * 128
            # gather idx layout: [128, L//16], [p, s] = seg[s*16 + p%16]
            idx_parts[c].append(
                np.tile(seg_i.reshape(L // 16, 16).T, (8, 1)).ravel())
            # rl/val layout: [128, L//128], [p, t] = seg[t*128 + p]
            rl_parts[c].append(seg_r.reshape(L // 128, 128).T.ravel())
            val_parts[c].append(seg_v.reshape(L // 128, 128).T.ravel())

    midx = [np.concatenate(p) for p in idx_parts]
    mrl = [np.concatenate(p) for p in rl_parts]
    mval = [np.concatenate(p) for p in val_parts]
    return calls, midx, mrl, mval


DEBUG_MODE = "full"   # "l1" = layer1 only no AG; "l1ag" = layer1 + AG; "full"


def _build_program(calls):
    import concourse.bacc as bacc
    import concourse.tile as tile
    import concourse.mybir as mybir

    idx_total = sum(128 * (L // 16) for (_, _, L, _) in calls)
    rv_total = sum(L for (_, _, L, _) in calls)

    nc = bacc.Bacc("TRN2", target_bir_lowering=False, debug=False,
                   enable_asserts=False, num_devices=N_CORES)
    f32 = mybir.dt.float32
    x0 = nc.dram_tensor("x0", [N, D], f32, kind="ExternalInput").ap()
    ego_sh = nc.dram_tensor("ego_sh", [NSH, D], f32, kind="ExternalInput").ap()
    midx = nc.dram_tensor("midx", [idx_total], mybir.dt.int16,
                          kind="ExternalInput").ap()
    mrl = nc.dram_tensor("mrl", [rv_total], f32, kind="ExternalInput").ap()
    mval = nc.dram_tensor("mval", [rv_total], f32, kind="ExternalInput").ap()
    out = nc.dram_tensor("out", [NSH, D], f32, kind="ExternalOutput").ap()

    with tile.TileContext(nc) as tc:
        with (
            tc.tile_pool(name="const", bufs=1) as constp,
            tc.tile_pool(name="meta", bufs=8) as meta,
            tc.tile_pool(name="msgs", bufs=8) as msgsp,
            tc.tile_pool(name="oh", bufs=6) as ohp,
            tc.tile_pool(name="fl", bufs=6) as flp,
            tc.tile_pool(name="psum", bufs=8, space="PSUM") as psump,
            tc.tile_pool(name="dram", bufs=1, space="DRAM") as dram,
        ):
            iota_i = constp.tile([128, 128], mybir.dt.int32)
            iota_f = constp.tile([128, 128], f32)
            nc.gpsimd.iota(iota_i[:], pattern=[[1, 128]], base=0,
                           channel_multiplier=0)
            nc.vector.tensor_copy(iota_f[:], iota_i[:])

            e_sh = dram.tile([NSH, D], f32)
            e_full = dram.tile([N, D], f32)

            n_layers = 1 if DEBUG_MODE in ("l1", "l1ag") else N_LAYERS
            for layer in range(n_layers):
                table = x0 if layer == 0 else e_full[:]
                ioff = 0
                rvoff = 0
                ci = 0
                for g in range(NG):
                    ws = list(range(g * GRP, min((g + 1) * GRP, NW)))
                    per_b = []
                    for b in range(NB):
                        (_, _, L, wts) = calls[ci + b]
                        S16 = L // 16
                        T = L // 128
                        idx_t = meta.tile([128, S16], mybir.dt.int16, tag="idx")
                        nc.sync.dma_start(
                            idx_t[:],
                            midx[ioff:ioff + 128 * S16].rearrange(
                                "(p s) -> p s", p=128))
                        rl_t = meta.tile([128, T], f32, tag="rl")
                        val_t = meta.tile([128, T], f32, tag="val")
                        nc.sync.dma_start(
                            rl_t[:],
                            mrl[rvoff:rvoff + L].rearrange("(p t) -> p t", p=128))
                        nc.sync.dma_start(
                            val_t[:],
                            mval[rvoff:rvoff + L].rearrange("(p t) -> p t", p=128))
                        m_t = msgsp.tile([128, T, D], f32, tag="m")
                        nc.gpsimd.dma_gather(
                            m_t[:], table[b * BLK:(b + 1) * BLK, :], idx_t[:],
                            L, L, D, single_packet=False)
                        per_b.append((m_t, rl_t, val_t, wts))
                        ioff += 128 * S16
                        rvoff += L
                    ci += NB

                    psums = {}
                    for w in ws:
                        ps_t = psump.tile([128, D], f32, tag="ps")
                        psums[w] = ps_t
                    # w-major: each window's PSUM accumulation group is
                    # contiguous in PE issue order (interleaved start/stop
                    # groups crash the device)
                    for wi, w in enumerate(ws):
                        first = True
                        for b in range(NB):
                            m_t, rl_t, val_t, wts = per_b[b]
                            toff = sum(nt for (_, nt) in wts[:wi])
                            nt = wts[wi][1]
                            for t in range(toff, toff + nt):
                                oh_t = ohp.tile([128, 128], f32, tag="oh")
                                nc.vector.tensor_scalar(
                                    oh_t[:], iota_f[:], rl_t[:, t:t + 1],
                                    val_t[:, t:t + 1],
                                    op0=mybir.AluOpType.is_equal,
                                    op1=mybir.AluOpType.mult)
                                stop = (b == NB - 1) and (t == toff + nt - 1)
                                nc.tensor.matmul(
                                    psums[w][:], oh_t[:], m_t[:, t, :],
                                    start=first, stop=stop)
                                first = False

                    for w in ws:
                        nrows = min(WIN, NSH - w * WIN)
                        if layer == 0:
                            fl_t = flp.tile([128, D], f32, tag="fl")
                            nc.scalar.copy(fl_t[:], psums[w][:])
                            nc.sync.dma_start(
                                e_sh[w * WIN:w * WIN + nrows, :],
                                fl_t[:nrows, :])
                            if DEBUG_MODE in ("l1", "l1ag"):
                                nc.sync.dma_start(
                                    out[w * WIN:w * WIN + nrows, :],
                                    fl_t[:nrows, :])
                        else:
                            ego_t = flp.tile([128, D], f32, tag="ego")
                            e1_t = flp.tile([128, D], f32, tag="e1")
                            nc.sync.dma_start(
                                ego_t[:nrows, :],
                                ego_sh[w * WIN:w * WIN + nrows, :])
                            nc.sync.dma_start(
                                e1_t[:nrows, :],
                                e_sh[w * WIN:w * WIN + nrows, :])
                            s_t = flp.tile([128, D], f32, tag="s")
                            nc.vector.tensor_tensor(
                                s_t[:], psums[w][:], e1_t[:],
                                op=mybir.AluOpType.add)
                            nc.vector.tensor_tensor(
                                s_t[:], s_t[:], ego_t[:],
                                op=mybir.AluOpType.add)
                            o_t = flp.tile([128, D], f32, tag="o")
                            nc.scalar.mul(o_t[:], s_t[:], 1.0 / (N_LAYERS + 1))
                            nc.sync.dma_start(
                                out[w * WIN:w * WIN + nrows, :],
                                o_t[:nrows, :])

                if layer == 0 and DEBUG_MODE != "l1":
                    nc.gpsimd.collective_compute(
                        "AllGather",
                        mybir.AluOpType.bypass,
                        replica_groups=[list(range(N_CORES))],
                        ins=[e_sh.opt()],
                        outs=[e_full.opt()],
                    )

    nc.compile()
    return nc


def kernel(sym_emb, herb_emb, adj_rows, adj_cols, adj_vals,
           _trace=False, _tmpdir=None):
    from concourse.bass_utils import run_bass_kernel_spmd

    sym_emb = np.asarray(sym_emb, np.float32)
    herb_emb = np.asarray(herb_emb, np.float32)
    rows = np.asarray(adj_rows).astype(np.int64)
    cols = np.asarray(adj_cols).astype(np.int64)
    vals = np.asarray(adj_vals, np.float32)

    ego = np.concatenate([sym_emb, herb_emb], axis=0)
    calls, midx, mrl, mval = _preprocess(rows, cols, vals)
    nc = _build_program(calls)

    in_maps = []
    for c in range(N_CORES):
        in_maps.append({
            "x0": ego,
            "ego_sh": ego[c * NSH:(c + 1) * NSH],
            "midx": midx[c],
            "mrl": mrl[c],
            "mval": mval[c],
        })
    res = run_bass_kernel_spmd(nc, in_maps, core_ids=list(range(N_CORES)),
                               trace=_trace, tmpdir=_tmpdir)
    acc = np.concatenate([res.results[c]["out"] for c in range(N_CORES)],
                         axis=0)
    kernel.last_results = res
    return acc[:NUM_SYM], acc[NUM_SYM:]
